# revision 1
# baseline (speedup 1.0000x reference)
"""TRN2 Bass kernel for nn_DoubleGSOFTCrossAttnProcessor.

Strategy
--------
The GSOFT block-diagonal orthogonal transforms (Cayley maps of tiny [16,b,b]
parameter blocks) are linear, so they fold into the dense projection weights
on the host:

    q = q_scale * gsoft(gsoft(x, Pq_in) @ Wq.T, Pq_out)
      = x @ [BD(Q(Pq_in)) @ Wq.T @ BD(Q(Pq_out)) @ diag(q_scale)] = x @ Wq_eff

(same for k, v and the output projection; the bias is added on the host after
the device pass). The device kernel is then plain 8-head cross-attention with
effective weights, data-parallel over batch: 8 batch elements -> 8 NeuronCores,
weights replicated, no collectives.

Device kernel (per core, all matmuls in float32r = TF32-like, fp32 PSUM):
  - Q^T = Wq_eff^T @ x^T per 512-seq tile (features on partitions).
  - scores^T[s_k, s_q] per head; softmax without max-subtraction (scores are
    O(5), exp can't overflow fp32): exp on ScalarE, key-sum via ones-matmul,
    reciprocal on VectorE, partition-broadcast via ones-matmul, normalize in
    place. Heads flow through a depth-3 software pipeline, and the previous
    tile's output-projection matmul groups are interleaved between the
    pipeline's dependent links as PE gap fillers.
  - attnout^T = V_h^T @ probs^T, evicted into a feature-permuted layout
    (HEAD_PERM) so every head's 160 features land 128-aligned.
  - out = attnout^T.T @ Wout_eff per 128-row seq chunk, DMA'd out.

HEAD_PERM: head h's first 128 score/value features -> chunk h; its last 32
packed into chunks 8-9 at row 32*(h%4). Applied to Wq/Wk columns, Wv columns
and Wout rows on the host, which makes every matmul operand and PSUM eviction
partition-aligned (the 160-dim head size is otherwise hostile to the
128-partition PE geometry).
"""


import numpy as np
from contextlib import ExitStack

import concourse.bass as bass
import concourse.bass_isa as bass_isa
import concourse.tile as tile
from concourse import bacc, mybir

F32 = mybir.dt.float32
F32R = mybir.dt.float32r

HID, CROSS, NBLK, HEADS = 1280, 768, 16, 8
HEAD_DIM = HID // HEADS               # 160
ATTN_SCALE = HEAD_DIM ** -0.5
SEQ, SKEY = 4096, 77
SKP = 80                              # padded key count (even, f32r requirement)
SQ = 512                              # seq-tile size
NT = SEQ // SQ                        # 8 seq tiles
KH, KC = HID // 128, CROSS // 128     # 10, 6 contraction chunks
XH = KH * SQ // 2                     # xt half-tile free size (2560)
NTILES = [(0, 512), (512, 512), (1024, 256)]  # featout tiles


def _cayley(P):
    P = P.astype(np.float64)
    A = P - np.swapaxes(P, -1, -2)
    I = np.eye(P.shape[-1], dtype=np.float64)
    return np.linalg.solve(I[None] - A, np.broadcast_to(I, A.shape) + A)


def _fold(P_in, W, P_out, scale):
    """W_eff = BD(Q_in) @ W.T @ BD(Q_out) @ diag(scale); W is [out, in]."""
    Qi, Qo = _cayley(P_in), _cayley(P_out)
    WT = W.astype(np.float64).T
    g, b = Qi.shape[0], Qi.shape[1]
    T1 = np.einsum("gij,gjc->gic", Qi, WT.reshape(g, b, -1)).reshape(WT.shape)
    go, bo = Qo.shape[0], Qo.shape[1]
    T2 = np.einsum("rgi,gij->rgj", T1.reshape(-1, go, bo), Qo).reshape(WT.shape)
    return T2 * scale.astype(np.float64)[None, :]


def _head_perm():
    """head h's first 128 features -> chunk h; last 32 -> chunk 8/9 row 32*(h%4)."""
    perm = np.empty(HID, np.int64)
    for h in range(HEADS):
        perm[128 * h : 128 * h + 128] = np.arange(160 * h, 160 * h + 128)
        perm[1024 + 32 * h : 1024 + 32 * h + 32] = np.arange(
            160 * h + 128, 160 * h + 160)
    return perm


HEAD_PERM = _head_perm()


def fold_weights(inputs):
    wq = _fold(inputs["Pq_in"], inputs["Wq"], inputs["Pq_out"], inputs["q_scale"])
    wk = _fold(inputs["Pk_in"], inputs["Wk"], inputs["Pk_out"], inputs["k_scale"])
    wv = _fold(inputs["Pv_in"], inputs["Wv"], inputs["Pv_out"], inputs["v_scale"])
    wo = _fold(inputs["Pout_in"], inputs["Wout"], inputs["Pout_out"],
               inputs["out_scale"])
    wq = wq[:, HEAD_PERM]
    wk = wk[:, HEAD_PERM]
    wv = wv[:, HEAD_PERM]
    wo = wo[HEAD_PERM, :]
    return (wq.astype(np.float32), wk.astype(np.float32),
            wv.astype(np.float32), wo.astype(np.float32))


def _pack_w(W):  # [K*128, M] -> [128, K*M]
    Kc = W.shape[0] // 128
    return np.ascontiguousarray(
        W.reshape(Kc, 128, W.shape[1]).transpose(1, 0, 2).reshape(128, -1))


def make_in_map(x_b, enc_b, wq, wk, wv, wo):
    xt = (x_b.T.reshape(KH, 128, NT, SQ).transpose(2, 1, 0, 3)
          .reshape(NT, 128, 2, XH).transpose(0, 2, 1, 3))
    xt = np.ascontiguousarray(xt)                    # [NT, 2, 128, XH]
    encp = np.zeros((SKP, CROSS), np.float32)
    encp[:SKEY] = enc_b
    enct = _pack_w(np.ascontiguousarray(encp.T))
    return {
        "xt": xt,
        "wq": _pack_w(wq), "wk": _pack_w(wk), "wv": _pack_w(wv), "wo": _pack_w(wo),
        "enct": enct,
        "ones": np.ones((128, SKP), np.float32),
    }


def _head_pieces(h):
    return [(h, 0, 128), (8 + h // 4, 32 * (h % 4), 32)]


def build_nc(loop_reps=1):
    nc = bacc.Bacc("TRN2", target_bir_lowering=False, debug=False)
    xt_d = nc.dram_tensor("xt", [NT, 2, 128, XH], F32R, kind="ExternalInput").ap()
    wq_d = nc.dram_tensor("wq", [128, KH * HID], F32R, kind="ExternalInput").ap()
    wk_d = nc.dram_tensor("wk", [128, KC * HID], F32R, kind="ExternalInput").ap()
    wv_d = nc.dram_tensor("wv", [128, KC * HID], F32R, kind="ExternalInput").ap()
    wo_d = nc.dram_tensor("wo", [128, KH * HID], F32R, kind="ExternalInput").ap()
    enct_d = nc.dram_tensor("enct", [128, KC * SKP], F32R, kind="ExternalInput").ap()
    ones_d = nc.dram_tensor("ones", [128, SKP], F32R, kind="ExternalInput").ap()
    out_d = nc.dram_tensor("out", [SEQ, HID], F32, kind="ExternalOutput").ap()

    with tile.TileContext(nc) as tc:
        with ExitStack() as ctx:
            ctx.enter_context(nc.allow_low_precision(
                "f32r matmul inputs; accumulation stays f32 in PSUM"))
            const = ctx.enter_context(tc.tile_pool(name="const", bufs=1))
            # order matters: wq + first xt halves first so B(0) starts early
            wq_t = const.tile([128, KH * HID], F32R, name="wq_t")
            nc.sync.dma_start(wq_t[:], wq_d)
            ones_t = const.tile([128, SKP], F32R, name="ones_t")
            nc.sync.dma_start(ones_t[:], ones_d)
            kt_t = const.tile([128, KH * SKP], F32R, name="kt_t")
            v_t = const.tile([128, HID], F32R, name="v_t")
            wo_t = const.tile([128, KH * HID], F32R, name="wo_t")

            xt_pool = ctx.enter_context(tc.tile_pool(name="xt", bufs=2))
            qt_pool = ctx.enter_context(tc.tile_pool(name="qt", bufs=1))
            psum_mm = ctx.enter_context(
                tc.tile_pool(name="psum_mm", bufs=2, space="PSUM"))

            if loop_reps > 1:
                # hint_engines: the ~2900-inst body exceeds IRAM blocks, so
                # prefetch the back-edge target (else ~4us I$ miss/iteration
                # inflates the measured per-pass slope)
                ctx.enter_context(tc.For_i(
                    0, loop_reps, 1,
                    hint_engines=(mybir.EngineType.PE, mybir.EngineType.DVE,
                                  mybir.EngineType.Activation,
                                  mybir.EngineType.SP, mybir.EngineType.Pool)))

            qt_tiles = {}

            def phase_B(t):
                xh = []
                for hf in range(2):
                    xx = xt_pool.tile([128, XH], F32R, tag="xt", name=f"xt{t}_{hf}")
                    nc.sync.dma_start(xx[:], xt_d[t, hf])
                    xh.append(xx)
                qt_t = qt_pool.tile([128, KH * SQ], F32R, tag="qt", name=f"qt{t}")
                for m in range(KH):
                    pq = psum_mm.tile([128, SQ], F32, tag="mm", name=f"pq{t}_{m}")
                    for k in range(KH):
                        nc.tensor.matmul(
                            pq[:],
                            wq_t[:, k * HID + m * 128 : k * HID + (m + 1) * 128],
                            xh[k // 5][:, (k % 5) * SQ : (k % 5 + 1) * SQ],
                            start=(k == 0), stop=(k == KH - 1),
                        )
                    nc.vector.tensor_copy(qt_t[:, m * SQ : (m + 1) * SQ], pq[:])
                qt_tiles[t] = qt_t

            phase_B(0)

            # ------- setup: KT = Wk_eff^T @ enc^T, V = enc @ Wv_eff (after B0)
            with tc.tile_pool(name="setup_e", bufs=1) as setup_e, \
                 tc.tile_pool(name="psum_setup", bufs=2, space="PSUM") as psum_s:
                enct_t = setup_e.tile([128, KC * SKP], F32R, name="enct_t")
                nc.sync.dma_start(enct_t[:], enct_d)
                with tc.tile_pool(name="setup_k", bufs=1) as setup_k:
                    wk_t = setup_k.tile([128, KC * HID], F32R, name="wk_t")
                    nc.sync.dma_start(wk_t[:], wk_d)
                    for m in range(KH):
                        pk = psum_s.tile([128, SKP], F32, tag="pk", name=f"pk{m}")
                        for k in range(KC):
                            nc.tensor.matmul(
                                pk[:],
                                wk_t[:, k * HID + m * 128 : k * HID + (m + 1) * 128],
                                enct_t[:, k * SKP : (k + 1) * SKP],
                                start=(k == 0), stop=(k == KC - 1),
                            )
                        nc.vector.tensor_copy(kt_t[:, m * SKP : (m + 1) * SKP], pk[:])
                with tc.tile_pool(name="setup_v", bufs=1) as setup_v:
                    wv_t = setup_v.tile([128, KC * HID], F32R, name="wv_t")
                    nc.sync.dma_start(wv_t[:], wv_d)
                    for (n_off, n_sz) in NTILES:
                        pv = psum_s.tile([SKEY, n_sz], F32, tag="pk", name=f"pv{n_off}")
                        for k in range(KC):
                            nc.tensor.matmul(
                                pv[:],
                                enct_t[:, k * SKP : k * SKP + SKEY],
                                wv_t[:, k * HID + n_off : k * HID + n_off + n_sz],
                                start=(k == 0), stop=(k == KC - 1),
                            )
                        nc.vector.tensor_copy(v_t[0:SKEY, n_off : n_off + n_sz], pv[:])

            # wo arrives while B(0)/setup computes
            nc.sync.dma_start(wo_t[:], wo_d)

            # ------- main pools (reuse the setup space)
            ot_pool = ctx.enter_context(tc.tile_pool(name="ot", bufs=2))
            exp_pool = ctx.enter_context(tc.tile_pool(name="exp", bufs=3))
            rc_pool = ctx.enter_context(tc.tile_pool(name="rc", bufs=2))
            out_pool = ctx.enter_context(tc.tile_pool(name="outsb", bufs=1))
            psum_at = ctx.enter_context(
                tc.tile_pool(name="psum_at", bufs=4, space="PSUM"))
            psum_av = ctx.enter_context(
                tc.tile_pool(name="psum_av", bufs=2, space="PSUM"))

            ot_tiles = {}

            def d_group_makers(t):
                """D-phase of tile t as a list of closures (12 matmul groups,
                store after each 128-row chunk's last group)."""
                ot_t = ot_tiles.pop(t)
                sbs = {}
                makers = []

                def mk(j, n_off, n_sz):
                    def run():
                        if j not in sbs:
                            sbs[j] = out_pool.tile([128, HID], F32, tag="osb",
                                                   name=f"ob{t}_{j}")
                        po = psum_mm.tile([128, n_sz], F32, tag="mm",
                                          name=f"po{t}_{j}_{n_off}")
                        for c in range(KH):
                            nc.tensor.matmul(
                                po[:],
                                ot_t[:, c * SQ + j * 128 : c * SQ + (j + 1) * 128],
                                wo_t[:, c * HID + n_off : c * HID + n_off + n_sz],
                                start=(c == 0), stop=(c == KH - 1),
                            )
                        nc.vector.tensor_copy(sbs[j][:, n_off : n_off + n_sz], po[:])
                        if n_off == NTILES[-1][0]:
                            nc.sync.dma_start(
                                out_d[t * SQ + j * 128 : t * SQ + (j + 1) * 128, :],
                                sbs[j][:],
                            )
                    return run

                for j in range(SQ // 128):
                    for (n_off, n_sz) in NTILES:
                        makers.append(mk(j, n_off, n_sz))
                return makers

            def phase_C(t, fillers):
                """Attention with depth-3 head pipeline; `fillers` (D-groups of
                t-1) emitted between dependent links as PE gap fillers."""
                qt_t = qt_tiles.pop(t)
                ot_t = ot_pool.tile([128, KH * SQ], F32R, tag="ot", name=f"ot{t}")
                exp_tiles, rcs = {}, {}

                def fill(n=1):
                    for _ in range(n):
                        if fillers:
                            fillers.pop(0)()

                def stage1(h):  # scoresT + exp
                    sc = psum_at.tile([SKP, SQ], F32, tag="attn", name=f"sc{t}_{h}")
                    for i, (c, o, L) in enumerate(_head_pieces(h)):
                        nc.tensor.matmul(
                            sc[:],
                            kt_t[o : o + L, c * SKP : (c + 1) * SKP],
                            qt_t[o : o + L, c * SQ : (c + 1) * SQ],
                            start=(i == 0), stop=(i == 1),
                            tile_position=(o, 0),
                        )
                    exp_h = exp_pool.tile([SKEY, SQ], F32R, tag="exp", name=f"ex{t}_{h}")
                    nc.scalar.activation(
                        exp_h[:], sc[0:SKEY, :],
                        mybir.ActivationFunctionType.Exp, scale=ATTN_SCALE,
                    )
                    exp_tiles[h] = exp_h

                def stage2(h):  # key-sum + reciprocal
                    sm = psum_at.tile([1, SQ], F32, tag="attn", name=f"sm{t}_{h}")
                    nc.tensor.matmul(sm[:], ones_t[0:SKEY, 0:1], exp_tiles[h][:],
                                     start=True, stop=True)
                    rc = rc_pool.tile([1, SQ], F32R, tag="rc", name=f"rc{t}_{h}")
                    nc.vector.reciprocal(rc[:], sm[:])
                    rcs[h] = rc

                def stage34(h):  # bcast + normalize, fill, then attnout
                    bc = psum_at.tile([SKEY, SQ], F32, tag="attn", name=f"bc{t}_{h}")
                    nc.tensor.matmul(bc[:], ones_t[0:1, 0:SKEY], rcs.pop(h)[:],
                                     start=True, stop=True)
                    nc.vector.tensor_tensor(exp_tiles[h][:], exp_tiles[h][:],
                                            bc[:], mybir.AluOpType.mult)
                    fill()  # PE gap while DVE normalizes
                    exp_h = exp_tiles.pop(h)
                    for (c, o, L, pname) in [(h, 0, 128, "pa"),
                                             (8 + h // 4, 32 * (h % 4), 32, "pb")]:
                        pos = c * 128 + o
                        pa = psum_av.tile([L, SQ], F32, tag="att",
                                          name=f"{pname}{t}_{h}")
                        nc.tensor.matmul(pa[:], v_t[0:SKEY, pos : pos + L],
                                         exp_h[:], start=True, stop=True)
                        nc.vector.tensor_copy(
                            ot_t[o : o + L, c * SQ : (c + 1) * SQ], pa[:])

                for s in range(HEADS + 2):
                    if s < HEADS:
                        stage1(s)
                    fill()
                    if 0 <= s - 1 < HEADS:
                        stage2(s - 1)
                    fill()
                    if 0 <= s - 2 < HEADS:
                        stage34(s - 2)
                while fillers:
                    fillers.pop(0)()
                ot_tiles[t] = ot_t

            for t in range(NT):
                if t > 0:
                    phase_B(t)
                fillers = d_group_makers(t - 1) if t > 0 else []
                phase_C(t, fillers)
            for run in d_group_makers(NT - 1):
                run()

    nc.finalize()
    return nc


from concourse.bass_utils import run_bass_kernel_spmd

_NC_CACHE = {}


def _get_nc(loop_reps=1):
    if loop_reps not in _NC_CACHE:
        _NC_CACHE[loop_reps] = build_nc(loop_reps)
    return _NC_CACHE[loop_reps]


def kernel(**inputs):
    inputs = {k: np.asarray(v) for k, v in inputs.items()}
    wq, wk, wv, wo = fold_weights(inputs)
    x = inputs["hidden_states"].astype(np.float32, copy=False)
    enc = inputs["encoder_hidden_states"].astype(np.float32, copy=False)
    B = x.shape[0]
    in_maps = [make_in_map(x[b], enc[b], wq, wk, wv, wo) for b in range(B)]
    nc = _get_nc()
    res = run_bass_kernel_spmd(nc, in_maps, list(range(B)))
    bout = inputs["bout"].astype(np.float32, copy=False)
    return np.stack([res.results[b]["out"] + bout[None, :] for b in range(B)])



# revision 3
# speedup vs baseline: 1.0921x; 1.0921x over previous
"""TRN2 Bass kernel for nn_DoubleGSOFTCrossAttnProcessor.

Strategy
--------
The GSOFT block-diagonal orthogonal transforms (Cayley maps of tiny [16,b,b]
parameter blocks) are linear, so they fold into the dense projection weights
on the host:

    q = q_scale * gsoft(gsoft(x, Pq_in) @ Wq.T, Pq_out)
      = x @ [BD(Q(Pq_in)) @ Wq.T @ BD(Q(Pq_out)) @ diag(q_scale)] = x @ Wq_eff

(same for k, v and the output projection; the bias is added on the host after
the device pass). The K/V paths only touch the tiny encoder states (77x768),
so K^T and V are computed on the host too and shipped as constants; the device
kernel is plain 8-head cross-attention over the 4096-token query stream,
data-parallel over batch: 8 batch elements -> 8 NeuronCores, no collectives.

Device kernel (per core; fp32 PSUM accumulation everywhere):
  - Q^T = Wq_eff^T @ x^T per 512-seq tile; x and Wq are shipped bf16 (halves
    their DMA; q error ~0.4% which the 2e-2 gate tolerates easily), Q^T is
    evicted f32r. Wq is chunked m-major into 10 tiles so the first B-phase
    matmul group starts after ~2us of DMA instead of 20us; each seq tile's
    x is prefetched one tile ahead (bufs=4) so the B phase never stalls.
  - scores^T[s_k, s_q] per head (f32r); softmax without max-subtraction
    (scores are O(5), exp can't overflow fp32): exp on ScalarE. The attnout
    matmuls consume the *unnormalized* exp; normalization is folded into the
    PSUM eviction: den = partition_all_reduce(exp) on the idle Pool engine,
    1/den row via the single-instruction approx reciprocal on DVE, Pool
    partition_broadcast to 128 rows, and the eviction becomes a
    tensor_tensor multiply. This removes both softmax ones-matmuls from the
    PE (the old key-sum + partition-broadcast matmuls) and the separate
    probs-normalize pass on DVE.
  - attnout^T = V_h^T @ exp^T, evicted (normalized) into a feature-permuted
    layout (HEAD_PERM) so every head's 160 features land 128-aligned.
  - out = attnout^T.T @ Wout_eff per 128-row seq chunk, DMA'd out f32.
    The previous tile's output-projection matmul groups are interleaved
    between the attention pipeline's dependent links as PE gap fillers.

HEAD_PERM: head h's first 128 score/value features -> chunk h; its last 32
packed into chunks 8-9 at row 32*(h%4). Applied to Wq/Wk columns, Wv columns
and Wout rows on the host, which makes every matmul operand and PSUM eviction
partition-aligned (the 160-dim head size is otherwise hostile to the
128-partition PE geometry).
"""


import numpy as np
import ml_dtypes
from contextlib import ExitStack

import concourse.bass as bass
import concourse.bass_isa as bass_isa
import concourse.tile as tile
from concourse import bacc, mybir
from concourse.bass_isa import ReduceOp

F32 = mybir.dt.float32
F32R = mybir.dt.float32r
BF16 = mybir.dt.bfloat16

HID, CROSS, NBLK, HEADS = 1280, 768, 16, 8
HEAD_DIM = HID // HEADS               # 160
ATTN_SCALE = HEAD_DIM ** -0.5
SEQ, SKEY = 4096, 77
SKP = 80                              # padded key count (even, f32r requirement)
SQ = 512                              # seq-tile size
NT = SEQ // SQ                        # 8 seq tiles
KH = HID // 128                       # 10 contraction chunks
XH = KH * SQ // 2                     # xt half-tile free size (2560)
NTILES = [(0, 512), (512, 512), (1024, 256)]  # featout tiles


def _cayley(P):
    P = P.astype(np.float64)
    A = P - np.swapaxes(P, -1, -2)
    I = np.eye(P.shape[-1], dtype=np.float64)
    return np.linalg.solve(I[None] - A, np.broadcast_to(I, A.shape) + A)


def _fold(P_in, W, P_out, scale):
    """W_eff = BD(Q_in) @ W.T @ BD(Q_out) @ diag(scale); W is [out, in]."""
    Qi, Qo = _cayley(P_in), _cayley(P_out)
    WT = W.astype(np.float64).T
    g, b = Qi.shape[0], Qi.shape[1]
    T1 = np.einsum("gij,gjc->gic", Qi, WT.reshape(g, b, -1)).reshape(WT.shape)
    go, bo = Qo.shape[0], Qo.shape[1]
    T2 = np.einsum("rgi,gij->rgj", T1.reshape(-1, go, bo), Qo).reshape(WT.shape)
    return T2 * scale.astype(np.float64)[None, :]


def _head_perm():
    """head h's first 128 features -> chunk h; last 32 -> chunk 8/9 row 32*(h%4)."""
    perm = np.empty(HID, np.int64)
    for h in range(HEADS):
        perm[128 * h : 128 * h + 128] = np.arange(160 * h, 160 * h + 128)
        perm[1024 + 32 * h : 1024 + 32 * h + 32] = np.arange(
            160 * h + 128, 160 * h + 160)
    return perm


HEAD_PERM = _head_perm()


def fold_weights(inputs):
    wq = _fold(inputs["Pq_in"], inputs["Wq"], inputs["Pq_out"], inputs["q_scale"])
    wk = _fold(inputs["Pk_in"], inputs["Wk"], inputs["Pk_out"], inputs["k_scale"])
    wv = _fold(inputs["Pv_in"], inputs["Wv"], inputs["Pv_out"], inputs["v_scale"])
    wo = _fold(inputs["Pout_in"], inputs["Wout"], inputs["Pout_out"],
               inputs["out_scale"])
    wq = wq[:, HEAD_PERM]
    wk = wk[:, HEAD_PERM]
    wv = wv[:, HEAD_PERM]
    wo = wo[HEAD_PERM, :]
    return wq, wk, wv, wo  # float64 [in, out]


def _pack_w(W):  # [K*128, M] -> [128, K*M]
    Kc = W.shape[0] // 128
    return np.ascontiguousarray(
        W.reshape(Kc, 128, W.shape[1]).transpose(1, 0, 2).reshape(128, -1))


def make_in_map(x_b, enc_b, wq, wk, wv, wo):
    # x^T tiles, bf16: [NT, 2, 128, XH]
    xt = (x_b.T.reshape(KH, 128, NT, SQ).transpose(2, 1, 0, 3)
          .reshape(NT, 128, 2, XH).transpose(0, 2, 1, 3))
    xt = np.ascontiguousarray(xt.astype(ml_dtypes.bfloat16))
    # Wq m-major chunks, bf16: wqm[m][:, k*128:(k+1)*128] = wq[k-chunk, m-chunk]
    wqm = (wq.reshape(KH, 128, KH, 128).transpose(2, 1, 0, 3)
           .reshape(KH, 128, HID))
    wqm = np.ascontiguousarray(wqm.astype(ml_dtypes.bfloat16))
    # Host K^T and V (tiny): K = encp @ wk  [80, 1280]
    encp = np.zeros((SKP, CROSS), np.float64)
    encp[:SKEY] = enc_b
    K = encp @ wk                                    # [80, 1280]
    V = encp @ wv                                    # [80, 1280]
    kt = np.ascontiguousarray(
        K.T.reshape(KH, 128, SKP).transpose(1, 0, 2).reshape(128, KH * SKP)
    ).astype(np.float32)
    vt = np.zeros((128, HID), np.float32)
    vt[:SKP] = V.astype(np.float32)
    return {
        "xt": xt,
        "wqm": wqm,
        "kt": kt,
        "vt": vt,
        "wo": _pack_w(wo.astype(np.float32)),
    }


def _head_pieces(h):
    return [(h, 0, 128), (8 + h // 4, 32 * (h % 4), 32)]


def build_nc(loop_reps=1):
    nc = bacc.Bacc("TRN2", target_bir_lowering=False, debug=False)
    xt_d = nc.dram_tensor("xt", [NT, 2, 128, XH], BF16, kind="ExternalInput").ap()
    wqm_d = nc.dram_tensor("wqm", [KH, 128, HID], BF16, kind="ExternalInput").ap()
    kt_d = nc.dram_tensor("kt", [128, KH * SKP], F32R, kind="ExternalInput").ap()
    vt_d = nc.dram_tensor("vt", [128, HID], F32R, kind="ExternalInput").ap()
    wo_d = nc.dram_tensor("wo", [128, KH * HID], F32R, kind="ExternalInput").ap()
    out_d = nc.dram_tensor("out", [SEQ, HID], F32, kind="ExternalOutput").ap()

    with tile.TileContext(nc) as tc:
        with ExitStack() as ctx:
            ctx.enter_context(nc.allow_low_precision(
                "bf16/f32r matmul inputs; accumulation stays f32 in PSUM"))
            const = ctx.enter_context(tc.tile_pool(name="const", bufs=1))
            # first two wq chunks + kt/vt land before anything else so B(0)
            # and C(0) start early; the rest stream behind the first xt tiles
            wqm_t = []
            for m in range(KH):
                wqm_t.append(const.tile([128, HID], BF16, name=f"wqm{m}"))
            nc.sync.dma_start(wqm_t[0][:], wqm_d[0])
            nc.sync.dma_start(wqm_t[1][:], wqm_d[1])
            kt_t = const.tile([128, KH * SKP], F32R, name="kt_t")
            nc.sync.dma_start(kt_t[:], kt_d)
            v_t = const.tile([128, HID], F32R, name="v_t")
            nc.sync.dma_start(v_t[:], vt_d)
            wo_t = const.tile([128, KH * HID], F32R, name="wo_t")

            def dma_const_rest():
                for m in range(2, KH):
                    nc.sync.dma_start(wqm_t[m][:], wqm_d[m])
                nc.sync.dma_start(wo_t[:], wo_d)

            xt_pool = ctx.enter_context(tc.tile_pool(name="xt", bufs=4))
            qt_pool = ctx.enter_context(tc.tile_pool(name="qt", bufs=1))
            psum_mm = ctx.enter_context(
                tc.tile_pool(name="psum_mm", bufs=2, space="PSUM"))

            xt_tiles = {}

            def prefetch_xt(t):
                xh = []
                for hf in range(2):
                    xx = xt_pool.tile([128, XH], BF16, tag="xt", name=f"xt{t}_{hf}")
                    nc.sync.dma_start(xx[:], xt_d[t, hf])
                    xh.append(xx)
                xt_tiles[t] = xh

            if loop_reps > 1:
                dma_const_rest()
                # hint_engines: the ~2700-inst body exceeds IRAM blocks, so
                # prefetch the back-edge target (else ~4us I$ miss/iteration
                # inflates the measured per-pass slope)
                ctx.enter_context(tc.For_i(
                    0, loop_reps, 1,
                    hint_engines=(mybir.EngineType.PE, mybir.EngineType.DVE,
                                  mybir.EngineType.Activation,
                                  mybir.EngineType.SP, mybir.EngineType.Pool)))
                prefetch_xt(0)
                prefetch_xt(1)
            else:
                prefetch_xt(0)
                prefetch_xt(1)
                dma_const_rest()

            qt_tiles = {}

            def phase_B(t):
                if t + 1 < NT and t + 1 not in xt_tiles:
                    prefetch_xt(t + 1)
                xh = xt_tiles.pop(t)
                qt_t = qt_pool.tile([128, KH * SQ], F32R, tag="qt", name=f"qt{t}")
                for m in range(KH):
                    pq = psum_mm.tile([128, SQ], F32, tag="mm", name=f"pq{t}_{m}")
                    for k in range(KH):
                        nc.tensor.matmul(
                            pq[:],
                            wqm_t[m][:, k * 128 : (k + 1) * 128],
                            xh[k // 5][:, (k % 5) * SQ : (k % 5 + 1) * SQ],
                            start=(k == 0), stop=(k == KH - 1),
                        )
                    nc.vector.tensor_copy(qt_t[:, m * SQ : (m + 1) * SQ], pq[:])
                qt_tiles[t] = qt_t

            # ------- main pools
            ot_pool = ctx.enter_context(tc.tile_pool(name="ot", bufs=2))
            exp_pool = ctx.enter_context(tc.tile_pool(name="exp", bufs=3))
            den_pool = ctx.enter_context(tc.tile_pool(name="den", bufs=2))
            rd_pool = ctx.enter_context(tc.tile_pool(name="rd", bufs=2))
            rdb_pool = ctx.enter_context(tc.tile_pool(name="rdb", bufs=2))
            out_pool = ctx.enter_context(tc.tile_pool(name="outsb", bufs=1))
            psum_at = ctx.enter_context(
                tc.tile_pool(name="psum_at", bufs=2, space="PSUM"))
            psum_av = ctx.enter_context(
                tc.tile_pool(name="psum_av", bufs=4, space="PSUM"))

            ot_tiles = {}

            def d_group_makers(t):
                """D-phase of tile t as a list of closures (12 matmul groups,
                store after each 128-row chunk's last group)."""
                ot_t = ot_tiles.pop(t)
                sbs = {}
                makers = []

                def mk(j, n_off, n_sz):
                    def run():
                        if j not in sbs:
                            sbs[j] = out_pool.tile([128, HID], F32, tag="osb",
                                                   name=f"ob{t}_{j}")
                        po = psum_mm.tile([128, n_sz], F32, tag="mm",
                                          name=f"po{t}_{j}_{n_off}")
                        for c in range(KH):
                            nc.tensor.matmul(
                                po[:],
                                ot_t[:, c * SQ + j * 128 : c * SQ + (j + 1) * 128],
                                wo_t[:, c * HID + n_off : c * HID + n_off + n_sz],
                                start=(c == 0), stop=(c == KH - 1),
                            )
                        nc.vector.tensor_copy(sbs[j][:, n_off : n_off + n_sz], po[:])
                        if n_off == NTILES[-1][0]:
                            nc.sync.dma_start(
                                out_d[t * SQ + j * 128 : t * SQ + (j + 1) * 128, :],
                                sbs[j][:],
                            )
                    return run

                for j in range(SQ // 128):
                    for (n_off, n_sz) in NTILES:
                        makers.append(mk(j, n_off, n_sz))
                return makers

            def phase_C(t, fillers):
                """Attention with depth-3 head pipeline; `fillers` (D-groups of
                t-1) emitted between dependent links as PE gap fillers."""
                qt_t = qt_tiles.pop(t)
                ot_t = ot_pool.tile([128, KH * SQ], F32R, tag="ot", name=f"ot{t}")
                exp_tiles, pa_tiles, rdbs, rds = {}, {}, {}, {}

                def fill(n=1):
                    for _ in range(n):
                        if fillers:
                            fillers.pop(0)()

                def stage_sc(h):  # scoresT
                    sc = psum_at.tile([SKP, SQ], F32, tag="attn", name=f"sc{t}_{h}")
                    for i, (c, o, L) in enumerate(_head_pieces(h)):
                        nc.tensor.matmul(
                            sc[:],
                            kt_t[o : o + L, c * SKP : (c + 1) * SKP],
                            qt_t[o : o + L, c * SQ : (c + 1) * SQ],
                            start=(i == 0), stop=(i == 1),
                            tile_position=(o, 0),
                        )
                    exp_tiles[h] = sc  # placeholder: exp in stage_mid reads sc

                def stage_mid(h):  # exp, den (Pool), attnout matmuls, 1/den
                    sc = exp_tiles.pop(h)
                    exp_h = exp_pool.tile([SKEY, SQ], F32R, tag="exp",
                                          name=f"ex{t}_{h}")
                    nc.scalar.activation(
                        exp_h[:], sc[0:SKEY, :],
                        mybir.ActivationFunctionType.Exp, scale=ATTN_SCALE,
                    )
                    den = den_pool.tile([SKEY, SQ], F32, tag="den",
                                        name=f"dn{t}_{h}")
                    nc.gpsimd.partition_all_reduce(
                        den[:], exp_h[:], SKEY, ReduceOp.add)
                    for (c, o, L, pname) in [(h, 0, 128, "pa"),
                                             (8 + h // 4, 32 * (h % 4), 32, "pb")]:
                        pos = c * 128 + o
                        pa = psum_av.tile([L, SQ], F32, tag="att",
                                          name=f"{pname}{t}_{h}")
                        nc.tensor.matmul(pa[:], v_t[0:SKEY, pos : pos + L],
                                         exp_h[:], start=True, stop=True)
                        pa_tiles[(h, c, o, L)] = pa
                    rd = rd_pool.tile([1, SQ], F32, tag="rd", name=f"rd{t}_{h}")
                    nc.vector.reciprocal_approx_fast(rd[:], den[0:1, :])
                    rds[h] = rd

                def stage_out(h):  # Pool broadcast of 1/den, fused evictions
                    rdb = rdb_pool.tile([128, SQ], F32, tag="rdb",
                                        name=f"rb{t}_{h}")
                    nc.gpsimd.partition_broadcast(rdb[:], rds.pop(h)[:], 128)
                    fill()  # PE gap while Pool broadcasts
                    for (c, o, L) in _head_pieces(h):
                        pa = pa_tiles.pop((h, c, o, L))
                        nc.vector.tensor_tensor(
                            ot_t[o : o + L, c * SQ : (c + 1) * SQ], pa[:],
                            rdb[0:L, :], mybir.AluOpType.mult)

                for s in range(HEADS + 2):
                    if s < HEADS:
                        stage_sc(s)
                    fill()
                    if 0 <= s - 1 < HEADS:
                        stage_mid(s - 1)
                    fill()
                    if 0 <= s - 2 < HEADS:
                        stage_out(s - 2)
                while fillers:
                    fillers.pop(0)()
                ot_tiles[t] = ot_t

            for t in range(NT):
                phase_B(t)
                fillers = d_group_makers(t - 1) if t > 0 else []
                phase_C(t, fillers)
            for run in d_group_makers(NT - 1):
                run()

    nc.finalize()
    return nc


from concourse.bass_utils import run_bass_kernel_spmd

_NC_CACHE = {}


def _get_nc(loop_reps=1):
    if loop_reps not in _NC_CACHE:
        _NC_CACHE[loop_reps] = build_nc(loop_reps)
    return _NC_CACHE[loop_reps]


def kernel(**inputs):
    inputs = {k: np.asarray(v) for k, v in inputs.items()}
    wq, wk, wv, wo = fold_weights(inputs)
    x = inputs["hidden_states"].astype(np.float32, copy=False)
    enc = inputs["encoder_hidden_states"].astype(np.float64, copy=False)
    B = x.shape[0]
    in_maps = [make_in_map(x[b], enc[b], wq, wk, wv, wo) for b in range(B)]
    nc = _get_nc()
    res = run_bass_kernel_spmd(nc, in_maps, list(range(B)))
    bout = inputs["bout"].astype(np.float32, copy=False)
    return np.stack([res.results[b]["out"] + bout[None, :] for b in range(B)])


# revision 7
# speedup vs baseline: 1.5142x; 1.3865x over previous
"""TRN2 Bass kernel for nn_DoubleGSOFTCrossAttnProcessor.

Strategy
--------
The GSOFT block-diagonal orthogonal transforms (Cayley maps of tiny [16,b,b]
parameter blocks) are linear, so they fold into the dense projection weights
on the host:

    q = q_scale * gsoft(gsoft(x, Pq_in) @ Wq.T, Pq_out)
      = x @ [BD(Q(Pq_in)) @ Wq.T @ BD(Q(Pq_out)) @ diag(q_scale)] = x @ Wq_eff

(same for k, v and the output projection; the bias is added on the host after
the device pass). The K/V paths only touch the tiny encoder states (77x768),
so K^T and V are computed on the host too and shipped as constants; the device
kernel is plain 8-head cross-attention over the 4096-token query stream,
data-parallel over batch: 8 batch elements -> 8 NeuronCores, no collectives.

Device kernel (per core; fp32 PSUM accumulation everywhere):
  - Q^T = Wq_eff^T @ x^T per 512-seq tile; x and Wq are shipped bf16 (halves
    their DMA; q error ~0.4% which the 2e-2 gate tolerates easily), Q^T is
    evicted f32r. Wq is chunked m-major into 10 tiles so the first B-phase
    matmul group starts after ~2us of DMA instead of 20us; each seq tile's
    x is prefetched one tile ahead (bufs=4) so the B phase never stalls.
  - scores^T[s_k, s_q] per head (f32r); softmax without max-subtraction
    (scores are O(5), exp can't overflow fp32): exp on ScalarE. The attnout
    matmuls consume the *unnormalized* exp; normalization is folded into the
    PSUM eviction: den = partition_all_reduce(exp) on the idle Pool engine,
    1/den row via the single-instruction approx reciprocal on DVE, Pool
    partition_broadcast to 128 rows, and the eviction becomes a
    tensor_tensor multiply. This removes both softmax ones-matmuls from the
    PE (the old key-sum + partition-broadcast matmuls) and the separate
    probs-normalize pass on DVE.
  - attnout^T = V_h^T @ exp^T, evicted (normalized) into a feature-permuted
    layout (HEAD_PERM) so every head's 160 features land 128-aligned.
  - out = attnout^T.T @ Wout_eff per 128-row seq chunk, DMA'd out f32.
    The previous tile's output-projection matmul groups are interleaved
    between the attention pipeline's dependent links as PE gap fillers.

HEAD_PERM: head h's first 128 score/value features -> chunk h; its last 32
packed into chunks 8-9 at row 32*(h%4). Applied to Wq/Wk columns, Wv columns
and Wout rows on the host, which makes every matmul operand and PSUM eviction
partition-aligned (the 160-dim head size is otherwise hostile to the
128-partition PE geometry).
"""


import numpy as np
import ml_dtypes
from contextlib import ExitStack

import concourse.bass as bass
import concourse.bass_isa as bass_isa
import concourse.tile as tile
from concourse import bacc, mybir
from concourse.bass_isa import ReduceOp

F32 = mybir.dt.float32
F32R = mybir.dt.float32r
BF16 = mybir.dt.bfloat16

HID, CROSS, NBLK, HEADS = 1280, 768, 16, 8
HEAD_DIM = HID // HEADS               # 160
ATTN_SCALE = HEAD_DIM ** -0.5
SEQ, SKEY = 4096, 77
SKP = 80                              # padded key count (even, f32r requirement)
SQ = 512                              # seq-tile size
NT = SEQ // SQ                        # 8 seq tiles
KH = HID // 128                       # 10 contraction chunks
XH = KH * SQ // 2                     # xt half-tile free size (2560)
NTILES = [(0, 512), (512, 512), (1024, 256)]  # featout tiles


def _cayley(P):
    P = P.astype(np.float64)
    A = P - np.swapaxes(P, -1, -2)
    I = np.eye(P.shape[-1], dtype=np.float64)
    return np.linalg.solve(I[None] - A, np.broadcast_to(I, A.shape) + A)


def _fold(P_in, W, P_out, scale):
    """W_eff = BD(Q_in) @ W.T @ BD(Q_out) @ diag(scale); W is [out, in]."""
    Qi, Qo = _cayley(P_in), _cayley(P_out)
    WT = W.astype(np.float64).T
    g, b = Qi.shape[0], Qi.shape[1]
    T1 = np.einsum("gij,gjc->gic", Qi, WT.reshape(g, b, -1)).reshape(WT.shape)
    go, bo = Qo.shape[0], Qo.shape[1]
    T2 = np.einsum("rgi,gij->rgj", T1.reshape(-1, go, bo), Qo).reshape(WT.shape)
    return T2 * scale.astype(np.float64)[None, :]


def _head_perm():
    """head h's first 128 features -> chunk h; last 32 -> chunk 8/9 row 32*(h%4)."""
    perm = np.empty(HID, np.int64)
    for h in range(HEADS):
        perm[128 * h : 128 * h + 128] = np.arange(160 * h, 160 * h + 128)
        perm[1024 + 32 * h : 1024 + 32 * h + 32] = np.arange(
            160 * h + 128, 160 * h + 160)
    return perm


HEAD_PERM = _head_perm()


def fold_weights(inputs):
    wq = _fold(inputs["Pq_in"], inputs["Wq"], inputs["Pq_out"], inputs["q_scale"])
    wk = _fold(inputs["Pk_in"], inputs["Wk"], inputs["Pk_out"], inputs["k_scale"])
    wv = _fold(inputs["Pv_in"], inputs["Wv"], inputs["Pv_out"], inputs["v_scale"])
    wo = _fold(inputs["Pout_in"], inputs["Wout"], inputs["Pout_out"],
               inputs["out_scale"])
    wq = wq[:, HEAD_PERM]
    wk = wk[:, HEAD_PERM]
    wv = wv[:, HEAD_PERM]
    wo = wo[HEAD_PERM, :]
    return wq, wk, wv, wo  # float64 [in, out]


def _pack_w(W):  # [K*128, M] -> [128, K*M]
    Kc = W.shape[0] // 128
    return np.ascontiguousarray(
        W.reshape(Kc, 128, W.shape[1]).transpose(1, 0, 2).reshape(128, -1))


def make_in_map(x_b, enc_b, wq, wk, wv, wo):
    # x^T tiles, bf16: [NT, 2, 128, XH]
    xt = (x_b.T.reshape(KH, 128, NT, SQ).transpose(2, 1, 0, 3)
          .reshape(NT, 128, 2, XH).transpose(0, 2, 1, 3))
    xt = np.ascontiguousarray(xt.astype(ml_dtypes.bfloat16))
    # Wq m-major chunks, bf16: wqm[m][:, k*128:(k+1)*128] = wq[k-chunk, m-chunk]
    wqm = (wq.reshape(KH, 128, KH, 128).transpose(2, 1, 0, 3)
           .reshape(KH, 128, HID))
    wqm = np.ascontiguousarray(wqm.astype(ml_dtypes.bfloat16))
    # Host K^T and V (tiny): K = encp @ wk  [80, 1280]
    encp = np.zeros((SKP, CROSS), np.float64)
    encp[:SKEY] = enc_b
    K = encp @ wk                                    # [80, 1280]
    V = encp @ wv                                    # [80, 1280]
    kt = np.ascontiguousarray(
        K.T.reshape(KH, 128, SKP).transpose(1, 0, 2).reshape(128, KH * SKP)
    ).astype(np.float32)
    vt = np.zeros((128, HID), np.float32)
    vt[:SKP] = V.astype(np.float32)
    return {
        "xt": xt,
        "wqm": wqm,
        "kt": kt,
        "vt": vt,
        "wo": _pack_w(wo.astype(np.float32)),
    }


def _head_pieces(h):
    return [(h, 0, 128), (8 + h // 4, 32 * (h % 4), 32)]


def build_nc(loop_reps=1):
    nc = bacc.Bacc("TRN2", target_bir_lowering=False, debug=False)
    xt_d = nc.dram_tensor("xt", [NT, 2, 128, XH], BF16, kind="ExternalInput").ap()
    wqm_d = nc.dram_tensor("wqm", [KH, 128, HID], BF16, kind="ExternalInput").ap()
    kt_d = nc.dram_tensor("kt", [128, KH * SKP], F32R, kind="ExternalInput").ap()
    vt_d = nc.dram_tensor("vt", [128, HID], F32R, kind="ExternalInput").ap()
    wo_d = nc.dram_tensor("wo", [128, KH * HID], F32R, kind="ExternalInput").ap()
    out_d = nc.dram_tensor("out", [SEQ, HID], F32, kind="ExternalOutput").ap()

    with tile.TileContext(nc) as tc:
        with ExitStack() as ctx:
            ctx.enter_context(nc.allow_low_precision(
                "bf16/f32r matmul inputs; accumulation stays f32 in PSUM"))
            const = ctx.enter_context(tc.tile_pool(name="const", bufs=1))
            # first two wq chunks + kt/vt land before anything else so B(0)
            # and C(0) start early; the rest stream behind the first xt tiles
            wqm_t = []
            for m in range(KH):
                wqm_t.append(const.tile([128, HID], BF16, name=f"wqm{m}"))
            nc.sync.dma_start(wqm_t[0][:], wqm_d[0])
            kt_t = const.tile([128, KH * SKP], F32R, name="kt_t")
            v_t = const.tile([128, HID], F32R, name="v_t")
            wo_t = const.tile([128, KH * HID], F32R, name="wo_t")

            def dma_const_early():
                nc.sync.dma_start(wqm_t[1][:], wqm_d[1])
                nc.sync.dma_start(kt_t[:], kt_d)
                nc.sync.dma_start(v_t[:], vt_d)

            def dma_const_rest():
                for m in range(2, KH):
                    nc.sync.dma_start(wqm_t[m][:], wqm_d[m])
                nc.sync.dma_start(wo_t[:], wo_d)

            xt_pool = ctx.enter_context(tc.tile_pool(name="xt", bufs=4))
            qt_pool = ctx.enter_context(tc.tile_pool(name="qt", bufs=1))
            psum_mm = ctx.enter_context(
                tc.tile_pool(name="psum_mm", bufs=2, space="PSUM"))

            xt_tiles = {}

            def prefetch_xt(t):
                xh = []
                for hf in range(2):
                    xx = xt_pool.tile([128, XH], BF16, tag="xt", name=f"xt{t}_{hf}")
                    nc.sync.dma_start(xx[:], xt_d[t, hf])
                    xh.append(xx)
                xt_tiles[t] = xh

            if loop_reps > 1:
                dma_const_early()
                dma_const_rest()
                # hint_engines: the ~2700-inst body exceeds IRAM blocks, so
                # prefetch the back-edge target (else ~4us I$ miss/iteration
                # inflates the measured per-pass slope)
                ctx.enter_context(tc.For_i(
                    0, loop_reps, 1,
                    hint_engines=(mybir.EngineType.PE, mybir.EngineType.DVE,
                                  mybir.EngineType.Activation,
                                  mybir.EngineType.SP, mybir.EngineType.Pool)))
                prefetch_xt(0)
                prefetch_xt(1)
            else:
                prefetch_xt(0)
                dma_const_early()
                prefetch_xt(1)
                dma_const_rest()

            qt_tiles = {}

            def phase_B(t):
                if t + 1 < NT and t + 1 not in xt_tiles:
                    prefetch_xt(t + 1)
                xh = xt_tiles.pop(t)
                qt_t = qt_pool.tile([128, KH * SQ], F32R, tag="qt", name=f"qt{t}")
                for m in range(KH):
                    pq = psum_mm.tile([128, SQ], F32, tag="mm", name=f"pq{t}_{m}")
                    for k in range(KH):
                        nc.tensor.matmul(
                            pq[:],
                            wqm_t[m][:, k * 128 : (k + 1) * 128],
                            xh[k // 5][:, (k % 5) * SQ : (k % 5 + 1) * SQ],
                            start=(k == 0), stop=(k == KH - 1),
                        )
                    # evict on ScalarE: keeps DVE free and PSUM rotation prompt
                    nc.scalar.copy(qt_t[:, m * SQ : (m + 1) * SQ], pq[:])
                qt_tiles[t] = qt_t

            # ------- main pools
            ot_pool = ctx.enter_context(tc.tile_pool(name="ot", bufs=2))
            exp_pool = ctx.enter_context(tc.tile_pool(name="exp", bufs=3))
            den_pool = ctx.enter_context(tc.tile_pool(name="den", bufs=2))
            rd_pool = ctx.enter_context(tc.tile_pool(name="rd", bufs=2))
            rdb_pool = ctx.enter_context(tc.tile_pool(name="rdb", bufs=2))
            out_pool = ctx.enter_context(tc.tile_pool(name="outsb", bufs=1))
            psum_at = ctx.enter_context(
                tc.tile_pool(name="psum_at", bufs=2, space="PSUM"))
            psum_av = ctx.enter_context(
                tc.tile_pool(name="psum_av", bufs=4, space="PSUM"))

            ot_tiles = {}

            def d_group_makers(t):
                """D-phase of tile t as a list of closures (12 matmul groups,
                store after each 128-row chunk's last group)."""
                ot_t = ot_tiles.pop(t)
                sbs = {}
                makers = []

                def mk(j, n_off, n_sz):
                    def run():
                        if j not in sbs:
                            sbs[j] = out_pool.tile([128, HID], F32, tag="osb",
                                                   name=f"ob{t}_{j}")
                        po = psum_mm.tile([128, n_sz], F32, tag="mm",
                                          name=f"po{t}_{j}_{n_off}")
                        for c in range(KH):
                            nc.tensor.matmul(
                                po[:],
                                ot_t[:, c * SQ + j * 128 : c * SQ + (j + 1) * 128],
                                wo_t[:, c * HID + n_off : c * HID + n_off + n_sz],
                                start=(c == 0), stop=(c == KH - 1),
                            )
                        nc.scalar.copy(sbs[j][:, n_off : n_off + n_sz], po[:])
                        if n_off == NTILES[-1][0]:
                            nc.sync.dma_start(
                                out_d[t * SQ + j * 128 : t * SQ + (j + 1) * 128, :],
                                sbs[j][:],
                            )
                    return run

                for j in range(SQ // 128):
                    for (n_off, n_sz) in NTILES:
                        makers.append(mk(j, n_off, n_sz))
                return makers

            def phase_C(t, fillers):
                """Attention with depth-3 head pipeline; `fillers` (D-groups of
                t-1) emitted between dependent links as PE gap fillers."""
                qt_t = qt_tiles.pop(t)
                ot_t = ot_pool.tile([128, KH * SQ], F32R, tag="ot", name=f"ot{t}")
                exp_tiles, pa_tiles, rdbs, rds = {}, {}, {}, {}

                def fill(n=1):
                    for _ in range(n):
                        if fillers:
                            fillers.pop(0)()

                def stage_sc(h):  # scoresT
                    sc = psum_at.tile([SKP, SQ], F32, tag="attn", name=f"sc{t}_{h}")
                    for i, (c, o, L) in enumerate(_head_pieces(h)):
                        nc.tensor.matmul(
                            sc[:],
                            kt_t[o : o + L, c * SKP : (c + 1) * SKP],
                            qt_t[o : o + L, c * SQ : (c + 1) * SQ],
                            start=(i == 0), stop=(i == 1),
                            tile_position=(o, 0),
                        )
                    exp_tiles[h] = sc  # placeholder: exp in stage_mid reads sc

                def stage_mid(h):  # exp, den (Pool), attnout matmuls, 1/den
                    sc = exp_tiles.pop(h)
                    exp_h = exp_pool.tile([SKEY, SQ], F32R, tag="exp",
                                          name=f"ex{t}_{h}")
                    nc.scalar.activation(
                        exp_h[:], sc[0:SKEY, :],
                        mybir.ActivationFunctionType.Exp, scale=ATTN_SCALE,
                    )
                    den = den_pool.tile([SKEY, SQ], F32, tag="den",
                                        name=f"dn{t}_{h}")
                    nc.gpsimd.partition_all_reduce(
                        den[:], exp_h[:], SKEY, ReduceOp.add)
                    for (c, o, L, pname) in [(h, 0, 128, "pa"),
                                             (8 + h // 4, 32 * (h % 4), 32, "pb")]:
                        pos = c * 128 + o
                        pa = psum_av.tile([L, SQ], F32, tag="att",
                                          name=f"{pname}{t}_{h}")
                        nc.tensor.matmul(pa[:], v_t[0:SKEY, pos : pos + L],
                                         exp_h[:], start=True, stop=True)
                        pa_tiles[(h, c, o, L)] = pa
                    rd = rd_pool.tile([1, SQ], F32, tag="rd", name=f"rd{t}_{h}")
                    nc.vector.reciprocal_approx_fast(rd[:], den[0:1, :])
                    rds[h] = rd

                def stage_out(h):  # Pool broadcast of 1/den, fused evictions
                    rdb = rdb_pool.tile([128, SQ], F32, tag="rdb",
                                        name=f"rb{t}_{h}")
                    nc.gpsimd.partition_broadcast(rdb[:], rds.pop(h)[:], 128)
                    fill()  # PE gap while Pool broadcasts
                    for (c, o, L) in _head_pieces(h):
                        pa = pa_tiles.pop((h, c, o, L))
                        nc.vector.tensor_tensor(
                            ot_t[o : o + L, c * SQ : (c + 1) * SQ], pa[:],
                            rdb[0:L, :], mybir.AluOpType.mult)

                for s in range(HEADS + 2):
                    if s < HEADS:
                        stage_sc(s)
                    fill()
                    if 0 <= s - 1 < HEADS:
                        stage_mid(s - 1)
                    fill()
                    if 0 <= s - 2 < HEADS:
                        stage_out(s - 2)
                while fillers:
                    fillers.pop(0)()
                ot_tiles[t] = ot_t

            for t in range(NT):
                phase_B(t)
                fillers = d_group_makers(t - 1) if t > 0 else []
                phase_C(t, fillers)
            for run in d_group_makers(NT - 1):
                run()

    nc.finalize()
    return nc


from concourse.bass_utils import run_bass_kernel_spmd

_NC_CACHE = {}


def _get_nc(loop_reps=1):
    if loop_reps not in _NC_CACHE:
        _NC_CACHE[loop_reps] = build_nc(loop_reps)
    return _NC_CACHE[loop_reps]


def kernel(**inputs):
    inputs = {k: np.asarray(v) for k, v in inputs.items()}
    wq, wk, wv, wo = fold_weights(inputs)
    x = inputs["hidden_states"].astype(np.float32, copy=False)
    enc = inputs["encoder_hidden_states"].astype(np.float64, copy=False)
    B = x.shape[0]
    in_maps = [make_in_map(x[b], enc[b], wq, wk, wv, wo) for b in range(B)]
    nc = _get_nc()
    res = run_bass_kernel_spmd(nc, in_maps, list(range(B)))
    bout = inputs["bout"].astype(np.float32, copy=False)
    return np.stack([res.results[b]["out"] + bout[None, :] for b in range(B)])
